# revision 14
# baseline (speedup 1.0000x reference)
"""Single-head causal attention (B=8, T=2048, H=1024, D=64) on 8 TRN2 NeuronCores.

Data-parallel over batch: one batch element per core, no collectives.

Per core (everything transposed so contractions land on partitions):
  Host supplies xT bf16 pre-laid as [128, 8, T] (partition p, h-block hb, t)
  so every DMA descriptor moves one 32KB contiguous run per partition —
  the DMA engines are packet-rate-bound (~0.4us/descriptor/engine), so
  descriptor size sets input bandwidth. Weights pre-packed [128, 8, 192]
  ([Wk | Wq | Wv] per h-block) for the same reason.
  Packed projection: stationary [Wk | Wq] [128h, 128] -> psum rows 0..63 =
  kT, rows 64..127 = qT; q is DMA-shifted to partitions 0..63 so scores run
  in the default 128x128 PE mode with K=64 at base 0. vT computed separately
  and v natural rows recovered with bf16 DMA-transposes (keeps TensorE free
  of transpose-mode drains, which oscillate the HAM clock gate).
  Scores transposed: sT[kj, qi] = k[kj]·q[qi] (1/8 folded into Wq on host).
  exp on ScalarE (no max subtraction: scores bounded ~±4 here), causal
  block-wise, diagonal masked with gpsimd affine_select, pT in bf16.
  AV: oT[d, qi] += [v[kb] | ones].T @ pT[kb]; the ones column accumulates
  the softmax denominator in psum row 64 for free.
  Normalize per 1024-wide group right after it finishes: sums row -> DRAM ->
  [128, 8] reshape -> DVE reciprocal -> DRAM -> partition-broadcast DMA ->
  DVE multiply -> DMA out. Output is [64, T]; host transposes back.

Engine roles: sync = input DMAs + all utility DMAs; scalar = 2 input DMAs +
exp; vector = psum->sbuf copies + reciprocal + final multiply; gpsimd =
diagonal masks. Work is emitted in two 1024-column super-groups so all
engines pipeline across groups.
"""

import sys
from contextlib import ExitStack

if "/opt/trn_rl_repo" not in sys.path:
    sys.path.insert(0, "/opt/trn_rl_repo")

import numpy as np
import ml_dtypes

import concourse.bass as bass
import concourse.tile as tile
from concourse import bacc, mybir
from concourse.bass_utils import run_bass_kernel_spmd

B, T, H, D = 8, 2048, 1024, 64
N_CORES = 8
HB = H // 128  # 8 h-blocks
SG = 2  # two 1024-wide column super-groups
SGW = T // SG  # 1024
KB = T // 128  # 16 key blocks
KPG = KB // SG  # key blocks per super-group

F32 = mybir.dt.float32
BF16 = mybir.dt.bfloat16


def build_kernel():
    nc = bacc.Bacc("TRN2", target_bir_lowering=False, debug=False, num_devices=N_CORES)

    # xt: [partition, h-block, t]; w: [partition, h-block, 192] = [Wk|Wq|Wv]
    xt_d = nc.dram_tensor("xt", [128, HB, T], BF16, kind="ExternalInput").ap()
    w_d = nc.dram_tensor("w", [128, HB, 3 * D], BF16, kind="ExternalInput").ap()
    out_d = nc.dram_tensor("out", [D, T], F32, kind="ExternalOutput").ap()

    with tile.TileContext(nc) as tc:
        _build(tc, xt_d, w_d, out_d)

    nc.compile()
    return nc


def _build(tc, xt_d, w_d, out_d):
    nc = tc.nc
    ctx = ExitStack()
    singles = ctx.enter_context(tc.tile_pool(name="singles", bufs=1))
    pspool = ctx.enter_context(tc.tile_pool(name="pspool", bufs=3, space="PSUM"))
    opool = ctx.enter_context(tc.tile_pool(name="opool", bufs=1, space="PSUM"))
    ppool = ctx.enter_context(tc.tile_pool(name="ppool", bufs=1))
    npool = ctx.enter_context(tc.tile_pool(name="npool", bufs=2))

    # ---- DRAM scratch for the normalize reshape/broadcast ----
    sums_d = nc.dram_tensor("sums_d", [SG, SGW], F32).ap()
    recip_d = nc.dram_tensor("recip_d", [SG, SGW], F32).ap()

    # ---- input DMAs: weights first, then xt in h-block pairs ----
    w_s = singles.tile([128, HB, 3 * D], BF16)
    nc.sync.dma_start(out=w_s[:], in_=w_d[:])

    xt_s = singles.tile([128, HB, T], BF16)
    for i in range(4):
        eng = nc.sync if i % 2 == 0 else nc.scalar
        eng.dma_start(
            out=xt_s[:, 2 * i : 2 * i + 2, :], in_=xt_d[:, 2 * i : 2 * i + 2, :]
        )

    wkq = w_s[:, :, 0:128]  # [Wk | Wq] stationary halves
    wv = w_s[:, :, 128:192]

    kT = singles.tile([64, T], BF16)
    qhi = singles.tile([128, T], BF16)  # q at partitions 64..127
    qlo = singles.tile([64, T], BF16)  # q DMA-shifted to partitions 0..63
    vT = singles.tile([64, T], BF16)

    v_aug = singles.tile([128, KB, 65], BF16)
    nc.vector.memset(v_aug[:, :, 64:65], 1.0)

    oT_s = singles.tile([64, T], F32)
    pt = {}  # kb -> bf16 tile [128, T - kb*128]

    for sg in range(SG):
        g0 = sg * SGW
        cols = bass.ds(g0, SGW)

        # ---- packed k/q projection, two 512-wide halves ----
        for half in range(2):
            hcols = bass.ds(g0 + half * 512, 512)
            acc = pspool.tile([128, 512], F32, tag="ps", name=f"acc_kq_{sg}_{half}")
            for hb in range(HB):
                nc.tensor.matmul(
                    acc[:],
                    wkq[:, hb, :],
                    xt_s[:, hb, hcols],
                    start=(hb == 0),
                    stop=(hb == HB - 1),
                )
            nc.vector.tensor_copy(kT[:, hcols], acc[0:64, :])
            nc.vector.tensor_copy(qhi[64:128, hcols], acc[64:128, :])
        nc.sync.dma_start(out=qlo[:, cols], in_=qhi[64:128, cols])

        # ---- v projection ----
        for half in range(2):
            hcols = bass.ds(g0 + half * 512, 512)
            acc = pspool.tile([64, 512], F32, tag="ps", name=f"acc_v_{sg}_{half}")
            for hb in range(HB):
                nc.tensor.matmul(
                    acc[:],
                    wv[:, hb, :],
                    xt_s[:, hb, hcols],
                    start=(hb == 0),
                    stop=(hb == HB - 1),
                )
            nc.vector.tensor_copy(vT[:, hcols], acc[:])

        # ---- v natural rows via bf16 DMA transpose ----
        for kb in range(KPG * sg, KPG * (sg + 1)):
            vnat = npool.tile([128, 64], BF16, tag="vn", name=f"vnat_{kb}")
            nc.sync.dma_start(
                out=vnat[:], in_=vT[:, bass.ts(kb, 128)], transpose=True
            )
            nc.vector.tensor_copy(v_aug[:, kb, 0:64], vnat[:])

        # ---- scores + exp (+ diag mask) for this super-group ----
        n_kb_sg = KPG * (sg + 1)
        for kb in range(n_kb_sg):
            qi_lo = kb * 128
            c0 = max(qi_lo, g0)
            c1 = g0 + SGW
            if kb not in pt:
                pt[kb] = ppool.tile(
                    [128, T - qi_lo], BF16, tag=f"p{kb}", name=f"pt_{kb}"
                )
            s_ps = pspool.tile([128, c1 - c0], F32, tag="ps", name=f"s_{kb}_{sg}")
            # matmul slices must stay within one 512-wide psum bank
            for b0 in range(g0, g0 + SGW, 512):
                m0, m1 = max(c0, b0), b0 + 512
                if m0 >= m1:
                    continue
                nc.tensor.matmul(
                    s_ps[:, m0 - c0 : m1 - c0],
                    kT[:, bass.ts(kb, 128)],
                    qlo[:, m0:m1],
                    start=True,
                    stop=True,
                )
            nc.scalar.activation(
                out=pt[kb][:, c0 - qi_lo : c1 - qi_lo],
                in_=s_ps[:],
                func=mybir.ActivationFunctionType.Exp,
            )
            if sg == kb // KPG:
                # diagonal block: zero where kj (partition) > qi (free)
                nc.gpsimd.affine_select(
                    out=pt[kb][:, 0:128],
                    in_=pt[kb][:, 0:128],
                    compare_op=mybir.AluOpType.is_ge,
                    fill=0.0,
                    base=0,
                    pattern=[[1, 128]],
                    channel_multiplier=-1,
                )

        # ---- AV accumulation into this super-group's [65, 1024] psum ----
        oT_ps = opool.tile([65, SGW], F32, tag="o", name=f"oT_{sg}")
        for kb in range(n_kb_sg):
            qi_lo = kb * 128
            c0 = max(qi_lo, g0)
            for b0 in range(g0, g0 + SGW, 512):
                m0, m1 = max(c0, b0), b0 + 512
                if m0 >= m1:
                    continue
                nc.tensor.matmul(
                    oT_ps[:, m0 - g0 : m1 - g0],
                    v_aug[:, kb, :],
                    pt[kb][:, m0 - qi_lo : m1 - qi_lo],
                    start=(kb == 0),
                    stop=(kb == 4 * (b0 // 512) + 3),
                )

        # ---- normalize + store this super-group (utility DMAs on sync) ----
        # Tile does not track deps through DRAM (MANAGED_SPACES is SBUF/PSUM),
        # so the DRAM round-trips below are ordered with explicit semaphores.
        srow = npool.tile([65, SGW], F32, tag="srow", name=f"srow_{sg}")
        nc.vector.tensor_copy(srow[64:65, :], oT_ps[64:65, :])
        s16 = npool.tile([128, SGW // 128], F32, tag="s16", name=f"s16_{sg}")
        nsem = nc.alloc_semaphore(f"nrm{sg}")
        with tc.tile_critical():
            nc.gpsimd.dma_start(
                out=sums_d[sg : sg + 1, :], in_=srow[64:65, :]
            ).then_inc(nsem, 16)
            nc.gpsimd.wait_ge(nsem, 16)
            nc.gpsimd.dma_start(
                out=s16[:], in_=sums_d[sg, :].rearrange("(p f) -> p f", p=128)
            ).then_inc(nsem, 16)
            nc.gpsimd.wait_ge(nsem, 32)
        nc.vector.reciprocal(out=s16[:], in_=s16[:])
        rb = npool.tile([64, SGW], F32, tag="rb", name=f"rb_{sg}")
        rsrc = recip_d[sg : sg + 1, :]
        with tc.tile_critical():
            nc.gpsimd.dma_start(
                out=recip_d[sg, :].rearrange("(p f) -> p f", p=128), in_=s16[:]
            ).then_inc(nsem, 16)
            nc.gpsimd.wait_ge(nsem, 48)
            nc.gpsimd.dma_start(
                out=rb[:],
                in_=bass.AP(
                    tensor=rsrc.tensor,
                    offset=rsrc.offset,
                    ap=[[0, 64]] + list(rsrc.ap[1:]),
                ),
            ).then_inc(nsem, 16)
            nc.gpsimd.wait_ge(nsem, 64)
        nc.vector.tensor_mul(oT_s[:, cols], oT_ps[0:64, :], rb[:])
        nc.sync.dma_start(out=out_d[:, cols], in_=oT_s[:, cols])

    ctx.close()


_NC_CACHE = {}


def _get_nc():
    if "nc" not in _NC_CACHE:
        _NC_CACHE["nc"] = build_kernel()
    return _NC_CACHE["nc"]


def make_in_maps(x, Wk, Wq, Wv):
    bf16 = ml_dtypes.bfloat16
    x = np.asarray(x, dtype=np.float32)
    wq = np.asarray(Wq, dtype=np.float32) / np.sqrt(np.float32(D))
    wk = np.asarray(Wk, dtype=np.float32)
    wv = np.asarray(Wv, dtype=np.float32)
    # [H, 192] = [Wk | Wq | Wv], then -> [128, HB, 192] (h = hb*128 + p)
    w = np.concatenate([wk, wq, wv], axis=1).astype(bf16)
    w = np.ascontiguousarray(w.reshape(HB, 128, 3 * D).transpose(1, 0, 2))
    in_maps = []
    for b in range(B):
        xt = x[b].T.astype(bf16)  # [H, T]
        xt = np.ascontiguousarray(xt.reshape(HB, 128, T).transpose(1, 0, 2))
        in_maps.append({"xt": xt, "w": w})
    return in_maps


def kernel(x, Wk, Wq, Wv, **_ignored):
    nc = _get_nc()
    in_maps = make_in_maps(x, Wk, Wq, Wv)
    res = run_bass_kernel_spmd(nc, in_maps, core_ids=list(range(N_CORES)))
    out = np.stack([res.results[b]["out"].T for b in range(B)])
    return out.astype(np.float32)


if __name__ == "__main__":
    x = np.random.randn(B, T, H).astype(np.float32)
    s = 1.0 / np.sqrt(H)
    Wk = np.random.uniform(-s, s, (H, D)).astype(np.float32)
    Wq = np.random.uniform(-s, s, (H, D)).astype(np.float32)
    Wv = np.random.uniform(-s, s, (H, D)).astype(np.float32)
    out = kernel(x=x, Wk=Wk, Wq=Wq, Wv=Wv)
    print("out shape:", out.shape, "finite:", np.isfinite(out).all())


# revision 18
# speedup vs baseline: 1.0421x; 1.0421x over previous
"""Single-head causal attention (B=8, T=2048, H=1024, D=64) on 8 TRN2 NeuronCores.

Data-parallel over batch: one batch element per core, no collectives.

Per core (everything transposed so contractions land on partitions):
  Host supplies xT bf16 pre-laid as [128, 8, T] (partition p, h-block hb, t)
  so every DMA descriptor moves one 32KB contiguous run per partition —
  the DMA engines are packet-rate-bound (~0.4us/descriptor/engine), so
  descriptor size sets input bandwidth. Weights pre-packed [128, 8, 192]
  ([Wk | Wq | Wv] per h-block) for the same reason.
  Packed projection: stationary [Wk | Wq] [128h, 128] -> psum rows 0..63 =
  kT, rows 64..127 = qT; q is DMA-shifted to partitions 0..63 so scores run
  in the default 128x128 PE mode with K=64 at base 0. vT computed separately
  and v natural rows recovered with bf16 DMA-transposes (keeps TensorE free
  of transpose-mode drains, which oscillate the HAM clock gate).
  Scores transposed: sT[kj, qi] = k[kj]·q[qi] (1/8 folded into Wq on host).
  exp on ScalarE (no max subtraction: scores bounded ~±4 here), causal
  block-wise, diagonal masked with gpsimd affine_select, pT in bf16.
  AV: oT[d, qi] += [v[kb] | ones].T @ pT[kb]; the ones column accumulates
  the softmax denominator in psum row 64 for free.
  Normalize per 1024-wide group right after it finishes: sums row -> DRAM ->
  [128, 8] reshape -> DVE reciprocal -> DRAM -> partition-broadcast DMA ->
  DVE multiply -> DMA out. Output is [64, T]; host transposes back.

Engine roles: sync = input DMAs + all utility DMAs; scalar = 2 input DMAs +
exp; vector = psum->sbuf copies + reciprocal + final multiply; gpsimd =
diagonal masks. Work is emitted in two 1024-column super-groups so all
engines pipeline across groups.
"""

import sys
from contextlib import ExitStack

if "/opt/trn_rl_repo" not in sys.path:
    sys.path.insert(0, "/opt/trn_rl_repo")

import numpy as np
import ml_dtypes

import concourse.bass as bass
import concourse.tile as tile
from concourse import bacc, mybir
from concourse.bass_utils import run_bass_kernel_spmd

B, T, H, D = 8, 2048, 1024, 64
N_CORES = 8
HB = H // 128  # 8 h-blocks
SG = 2  # two 1024-wide column super-groups
SGW = T // SG  # 1024
KB = T // 128  # 16 key blocks
KPG = KB // SG  # key blocks per super-group

LINEARIZE = False
F32 = mybir.dt.float32
BF16 = mybir.dt.bfloat16


def build_kernel():
    nc = bacc.Bacc("TRN2", target_bir_lowering=False, debug=False, num_devices=N_CORES)

    # xt: [partition, h-block, t]; w: [partition, h-block, 192] = [Wk|Wq|Wv]
    xt_d = nc.dram_tensor("xt", [128, HB, T], BF16, kind="ExternalInput").ap()
    w_d = nc.dram_tensor("w", [128, HB, 3 * D], BF16, kind="ExternalInput").ap()
    out_d = nc.dram_tensor("out", [D, T], F32, kind="ExternalOutput").ap()

    with tile.TileContext(nc, linearize=LINEARIZE) as tc:
        _build(tc, xt_d, w_d, out_d)

    nc.compile()
    return nc


def _build(tc, xt_d, w_d, out_d):
    nc = tc.nc
    ctx = ExitStack()
    singles = ctx.enter_context(tc.tile_pool(name="singles", bufs=1))
    pspool = ctx.enter_context(tc.tile_pool(name="pspool", bufs=3, space="PSUM"))
    opool = ctx.enter_context(tc.tile_pool(name="opool", bufs=1, space="PSUM"))
    ppool = ctx.enter_context(tc.tile_pool(name="ppool", bufs=1))
    npool = ctx.enter_context(tc.tile_pool(name="npool", bufs=2))

    # ---- DRAM scratch for the normalize reshape/broadcast ----
    sums_d = nc.dram_tensor("sums_d", [SG, SGW], F32).ap()
    recip_d = nc.dram_tensor("recip_d", [SG, SGW], F32).ap()

    # ---- input DMAs: weights first, then xt in h-block pairs ----
    w_s = singles.tile([128, HB, 3 * D], BF16)
    nc.sync.dma_start(out=w_s[:], in_=w_d[:])

    xt_s = singles.tile([128, HB, T], BF16)
    for i in range(4):
        eng = nc.sync if i % 2 == 0 else nc.scalar
        eng.dma_start(
            out=xt_s[:, 2 * i : 2 * i + 2, :], in_=xt_d[:, 2 * i : 2 * i + 2, :]
        )

    wkq = w_s[:, :, 0:128]  # [Wk | Wq] stationary halves
    wv = w_s[:, :, 128:192]

    kT = singles.tile([64, T], BF16)
    qhi = singles.tile([128, T], BF16)  # q at partitions 64..127
    qlo = singles.tile([64, T], BF16)  # q DMA-shifted to partitions 0..63
    vT = singles.tile([64, T], BF16)

    v_aug = singles.tile([128, KB, 65], BF16)
    nc.vector.memset(v_aug[:, :, 64:65], 1.0)
    ident = singles.tile([64, 64], BF16)
    nc.gpsimd.memset(ident[:], 0.0)
    nc.gpsimd.affine_select(
        out=ident[:], in_=ident[:], compare_op=mybir.AluOpType.not_equal,
        fill=1.0, base=0, pattern=[[-1, 64]], channel_multiplier=1,
    )

    oT_s = singles.tile([64, T], F32)
    pt = {}  # kb -> bf16 tile [128, T - kb*128]

    for sg in range(SG):
        g0 = sg * SGW
        cols = bass.ds(g0, SGW)

        # ---- packed k/q projection, two 512-wide halves ----
        for half in range(2):
            hcols = bass.ds(g0 + half * 512, 512)
            acc = pspool.tile([128, 512], F32, tag="ps", name=f"acc_kq_{sg}_{half}")
            for hb in range(HB):
                nc.tensor.matmul(
                    acc[:],
                    wkq[:, hb, :],
                    xt_s[:, hb, hcols],
                    start=(hb == 0),
                    stop=(hb == HB - 1),
                )
            nc.vector.tensor_copy(kT[:, hcols], acc[0:64, :])
            nc.vector.tensor_copy(qhi[64:128, hcols], acc[64:128, :])
        nc.sync.dma_start(out=qlo[:, cols], in_=qhi[64:128, cols])

        # ---- v projection ----
        for half in range(2):
            hcols = bass.ds(g0 + half * 512, 512)
            acc = pspool.tile([64, 512], F32, tag="ps", name=f"acc_v_{sg}_{half}")
            for hb in range(HB):
                nc.tensor.matmul(
                    acc[:],
                    wv[:, hb, :],
                    xt_s[:, hb, hcols],
                    start=(hb == 0),
                    stop=(hb == HB - 1),
                )
            nc.vector.tensor_copy(vT[:, hcols], acc[:])

        # ---- v natural rows via PE transpose ----
        for kb in range(KPG * sg, KPG * (sg + 1)):
            vt_ps = pspool.tile([128, 64], BF16, tag="ps", name=f"vt_{kb}")
            nc.tensor.transpose(vt_ps[:], vT[:, bass.ts(kb, 128)], ident[:])
            nc.vector.tensor_copy(v_aug[:, kb, 0:64], vt_ps[:])

        # ---- scores + exp (+ diag mask) for this super-group ----
        n_kb_sg = KPG * (sg + 1)
        for kb in range(n_kb_sg):
            qi_lo = kb * 128
            c0 = max(qi_lo, g0)
            c1 = g0 + SGW
            if kb not in pt:
                pt[kb] = ppool.tile(
                    [128, T - qi_lo], BF16, tag=f"p{kb}", name=f"pt_{kb}"
                )
            # bank-aligned tile: matmul output slices must not cross the
            # 2KB psum bank boundary, so keep tile offsets == global offsets
            s_ps = pspool.tile([128, SGW], F32, tag="ps", name=f"s_{kb}_{sg}")
            for b0 in range(g0, g0 + SGW, 512):
                m0, m1 = max(c0, b0), b0 + 512
                if m0 >= m1:
                    continue
                nc.tensor.matmul(
                    s_ps[:, m0 - g0 : m1 - g0],
                    kT[:, bass.ts(kb, 128)],
                    qlo[:, m0:m1],
                    start=True,
                    stop=True,
                )
            nc.scalar.activation(
                out=pt[kb][:, c0 - qi_lo : c1 - qi_lo],
                in_=s_ps[:, c0 - g0 : SGW],
                func=mybir.ActivationFunctionType.Exp,
            )
            if sg == kb // KPG:
                # diagonal block: zero where kj (partition) > qi (free)
                nc.gpsimd.affine_select(
                    out=pt[kb][:, 0:128],
                    in_=pt[kb][:, 0:128],
                    compare_op=mybir.AluOpType.is_ge,
                    fill=0.0,
                    base=0,
                    pattern=[[1, 128]],
                    channel_multiplier=-1,
                )

        # ---- AV accumulation into this super-group's [65, 1024] psum ----
        oT_ps = opool.tile([65, SGW], F32, tag="o", name=f"oT_{sg}")
        for kb in range(n_kb_sg):
            qi_lo = kb * 128
            c0 = max(qi_lo, g0)
            for b0 in range(g0, g0 + SGW, 512):
                m0, m1 = max(c0, b0), b0 + 512
                if m0 >= m1:
                    continue
                nc.tensor.matmul(
                    oT_ps[:, m0 - g0 : m1 - g0],
                    v_aug[:, kb, :],
                    pt[kb][:, m0 - qi_lo : m1 - qi_lo],
                    start=(kb == 0),
                    stop=(kb == 4 * (b0 // 512) + 3),
                )

        # ---- normalize + store this super-group (utility DMAs on sync) ----
        # Tile does not track deps through DRAM (MANAGED_SPACES is SBUF/PSUM),
        # so the DRAM round-trips below are ordered with explicit semaphores.
        srow = npool.tile([65, SGW], F32, tag="srow", name=f"srow_{sg}")
        nc.vector.tensor_copy(srow[64:65, :], oT_ps[64:65, :])
        s16 = npool.tile([128, SGW // 128], F32, tag="s16", name=f"s16_{sg}")
        nsem = nc.alloc_semaphore(f"nrm{sg}")
        with tc.tile_critical():
            nc.gpsimd.dma_start(
                out=sums_d[sg : sg + 1, :], in_=srow[64:65, :]
            ).then_inc(nsem, 16)
            nc.gpsimd.wait_ge(nsem, 16)
            nc.gpsimd.dma_start(
                out=s16[:], in_=sums_d[sg, :].rearrange("(p f) -> p f", p=128)
            ).then_inc(nsem, 16)
            nc.gpsimd.wait_ge(nsem, 32)
        nc.vector.reciprocal(out=s16[:], in_=s16[:])
        rb = npool.tile([64, SGW], F32, tag="rb", name=f"rb_{sg}")
        rsrc = recip_d[sg : sg + 1, :]
        with tc.tile_critical():
            nc.gpsimd.dma_start(
                out=recip_d[sg, :].rearrange("(p f) -> p f", p=128), in_=s16[:]
            ).then_inc(nsem, 16)
            nc.gpsimd.wait_ge(nsem, 48)
            nc.gpsimd.dma_start(
                out=rb[:],
                in_=bass.AP(
                    tensor=rsrc.tensor,
                    offset=rsrc.offset,
                    ap=[[0, 64]] + list(rsrc.ap[1:]),
                ),
            ).then_inc(nsem, 16)
            nc.gpsimd.wait_ge(nsem, 64)
        nc.vector.tensor_mul(oT_s[:, cols], oT_ps[0:64, :], rb[:])
        nc.sync.dma_start(out=out_d[:, cols], in_=oT_s[:, cols])

    ctx.close()


_NC_CACHE = {}


def _get_nc():
    if "nc" not in _NC_CACHE:
        _NC_CACHE["nc"] = build_kernel()
    return _NC_CACHE["nc"]


def make_in_maps(x, Wk, Wq, Wv):
    bf16 = ml_dtypes.bfloat16
    x = np.asarray(x, dtype=np.float32)
    wq = np.asarray(Wq, dtype=np.float32) / np.sqrt(np.float32(D))
    wk = np.asarray(Wk, dtype=np.float32)
    wv = np.asarray(Wv, dtype=np.float32)
    # [H, 192] = [Wk | Wq | Wv], then -> [128, HB, 192] (h = hb*128 + p)
    w = np.concatenate([wk, wq, wv], axis=1).astype(bf16)
    w = np.ascontiguousarray(w.reshape(HB, 128, 3 * D).transpose(1, 0, 2))
    in_maps = []
    for b in range(B):
        xt = x[b].T.astype(bf16)  # [H, T]
        xt = np.ascontiguousarray(xt.reshape(HB, 128, T).transpose(1, 0, 2))
        in_maps.append({"xt": xt, "w": w})
    return in_maps


def kernel(x, Wk, Wq, Wv, **_ignored):
    nc = _get_nc()
    in_maps = make_in_maps(x, Wk, Wq, Wv)
    res = run_bass_kernel_spmd(nc, in_maps, core_ids=list(range(N_CORES)))
    out = np.stack([res.results[b]["out"].T for b in range(B)])
    return out.astype(np.float32)


if __name__ == "__main__":
    x = np.random.randn(B, T, H).astype(np.float32)
    s = 1.0 / np.sqrt(H)
    Wk = np.random.uniform(-s, s, (H, D)).astype(np.float32)
    Wq = np.random.uniform(-s, s, (H, D)).astype(np.float32)
    Wv = np.random.uniform(-s, s, (H, D)).astype(np.float32)
    out = kernel(x=x, Wk=Wk, Wq=Wq, Wv=Wv)
    print("out shape:", out.shape, "finite:", np.isfinite(out).all())
